# revision 57
# baseline (speedup 1.0000x reference)
"""Depth rasterization (MANO hand z-buffer @ 640x640 -> bilinear 128x128).

Key identities exploited:
  * jax.image.resize(640->128, linear, antialias=False) samples input coords
    5*j + 2.0 exactly -> output[i, j] == raster[5i+2, 5j+2]. Only the 128x128
    decimated pixel grid (centers x = 5j+2.5, y = 5i+2.5) is rasterized: a
    25x reduction vs the reference's 640x640 raster.
  * Edge functions and barycentric depth are affine in pixel coords, so each
    triangle yields four planes over the local basis (jl, il, 1) of its tile
    (tile origin folded into the constant term on host):
      P_k = OFF - S * sign(area) * e_k     (k = 0,1,2 penalty planes)
      W   = (e0*z0 + e1*z1 + e2*z2) / area (depth plane)
    key(p, f) = max(P0, P1, P2, W) equals the interpolated depth when p is
    inside triangle f and is >= OFF (>> the 100 clamp) outside; the z-buffer
    is zbuf(p) = min(100, min_f key(p, f)).
  * Plane evaluation is a K=9 bf16 matmul (coefficients split into 3 bf16
    limbs; the (jl, il, 1) basis is exact in bf16, giving fp32-grade accuracy
    at bf16 PE speed). Per slot of width w the PE writes [P0|W] (bank A,
    quadrant ra) and [P1|P2] (bank B, quadrant rb); ScalarE pulls bank A to
    SBUF (DVE reads at most one PSUM operand), DVE takes u = max(ta, pb)
    (bf16 out), pair-max within each slot (3D APs, one instr per group), and
    min-reduces over candidates (axis=X over [128, k, w]).
  * Per 16x8-pixel tile, candidates are bbox-filtered, then pruned exactly on
    an 8x8 subtile grid: a candidate is dropped if in every subtile it either
    misses the subtile entirely (some edge has all 4 corners outside by a
    margin safely above the reference's own fp32 edge-function noise) or its
    minimum possible depth exceeds the subtile's best fully-covering
    candidate's maximum depth. Exact vs the reference for any input.
  * Only the 9 coefficient rows actually read by the PE are shipped (HBM
    [9, cols]); the pixel basis is one shared [9, 128] tensor replicated into
    the 4 PE quadrants on device.

Sharding: 8 cores; (batch, tile) work items rank-balanced across all cores.
"""

import numpy as np
import ml_dtypes

import concourse.bacc as bacc
import concourse.mybir as mybir
import concourse.tile as tile
from concourse.bass_utils import run_bass_kernel_spmd

_B, _V, _F = 4, 778, 1538
_H = _W = 128
_TJ, _TI = 16, 8   # tile size in output pixels (x, y)
_NTILE = (_H // _TI) * (_W // _TJ)  # 128 tiles per batch image
_SUBJ, _SUBI = 8, 8  # subtile grid for host pruning
_WMAX = 256        # max slot width
_GRAN = 2          # slot width granularity (even: keeps 4B-aligned pair APs)
_OFF = 1000.0      # penalty-plane offset (>> 100 clamp)
_S = 1.0e9         # penalty scale
_BIGC = 1.0e7      # plane constant for padding/invalid
_CLAMP = 100.0
_COVER_MARGIN = 0.3    # e*s margin (e-units) for the full-cover test
_REJ_MARGIN = 0.3      # e*s margin (e-units) for exact edge rejection
_BOUND_MARGIN = 1e-3   # depth margin for the prune bound

_F32 = mybir.dt.float32
_BF16 = mybir.dt.bfloat16
_BF16_NP = ml_dtypes.bfloat16

_NC_CACHE = {}
PROFILE = {}


def _region_layout(sched):
    """sched: ((w, k, s0), ...) in device processing order. All matmuls run
    on PE quadrant (0,0): comp-A and comp-B both live in SBUF rows 0-8, so
    one HBM row-space [9, X] feeds everything and only one DMA strip gates
    the pipeline start. Columns: [g0 A|B] [g1 A|B] [pix] [g2 A|B] ...; the
    first strip ends after pix (covers groups 0-1 + the pixel basis)."""
    colbase = [0] * len(sched)
    o = 0
    for gi in range(min(2, len(sched))):
        colbase[gi] = o
        o += 4 * sched[gi][0] * sched[gi][1]
    pix = o
    o += 128
    strip1 = o
    for gi in range(2, len(sched)):
        colbase[gi] = o
        o += 4 * sched[gi][0] * sched[gi][1]
    return o, strip1, colbase, pix


def _build_nc(sched):
    """sched: ((w, k, s0), ...) slots padded to width w, k per group, with
    2*w*k <= 512 (one PSUM bank per component pair)."""
    nslot = sum(k for _, k, _ in sched)
    xt, strip1, colbase, pix = _region_layout(sched)
    nc = bacc.Bacc("TRN2", target_bir_lowering=False, debug=False, num_devices=8)
    data_d = nc.dram_tensor("data", [9, xt], _BF16, kind="ExternalInput")
    out_d = nc.dram_tensor("out", [128, nslot], _BF16, kind="ExternalOutput")

    with tile.TileContext(nc) as tc:
        with (
            tc.tile_pool(name="const", bufs=1) as cpool,
            tc.tile_pool(name="scr", bufs=6) as spool,
            tc.tile_pool(name="ps", bufs=4, space="PSUM") as ppool,
        ):
            ct = cpool.tile([128, xt], _BF16, name="ct")
            # 2 strip DMAs into rows 0-8; the first (groups 0-1 + pix) gates
            # the pipeline start, the second covers the rest. Each dma_start
            # costs ~0.7us of serial descriptor-write time on Sync.
            nc.sync.dma_start(ct[0:9, 0:strip1], data_d.ap()[0:9, 0:strip1])
            if strip1 < xt:
                nc.sync.dma_start(ct[0:9, strip1:xt], data_d.ap()[0:9, strip1:xt])
            # zmin columns in processing order (host maps back), split into
            # two tiles so the bulk output DMA's dependency resolves before
            # the last two (smallest) groups finish.
            z0 = 0
            zbase = []
            for w, k, s0 in sched:
                zbase.append(z0)
                z0 += k
            ztail = zbase[-1] if len(sched) > 1 else 0
            zmin_a = cpool.tile([128, max(ztail, 1)], _BF16, name="zmina")
            zmin_b = cpool.tile([128, nslot - ztail], _BF16, name="zminb")

            def zview(col0, k):
                if col0 >= ztail:
                    return zmin_b[:, col0 - ztail : col0 - ztail + k]
                return zmin_a[:, col0 : col0 + k]

            # dual-copy the largest groups: ScalarE pulls BOTH psum banks to
            # bf16 SBUF so their wide-max runs at DVE 2x (all-2B operands).
            # bf16(max) == max(bf16) exactly, so accuracy is unchanged.
            # Balance: each dual group adds ~0.61us scalar, saves ~0.33us DVE.
            dual = set()  # dual-copy measured as a wash: scalar serialization
                          # per group eats the DVE 2x win

            for gi, (w, k, s0) in enumerate(sched):
                wk = w * k
                og = colbase[gi]
                pa = ppool.tile([128, 2 * wk], _F32, tag="pa", name="pa")
                pb = ppool.tile([128, 2 * wk], _F32, tag="pb", name="pb")
                nc.tensor.matmul(pa[:, :], ct[0:9, pix : pix + 128],
                                 ct[0:9, og : og + 2 * wk],
                                 start=True, stop=True, tile_position=(0, 0))
                nc.tensor.matmul(pb[:, :], ct[0:9, pix : pix + 128],
                                 ct[0:9, og + 2 * wk : og + 4 * wk],
                                 start=True, stop=True, tile_position=(0, 0))
                # ScalarE pulls comp-A to SBUF (DVE reads max one PSUM operand)
                ta = spool.tile([128, 2 * wk], _F32, tag="ta", name="ta")
                nc.scalar.copy(ta[:, :], pa[:, :])
                u = spool.tile([128, 2 * wk], _BF16, tag="u", name="u")
                nc.vector.tensor_tensor(u[:, :], ta[:, :], pb[:, :],
                                        op=mybir.AluOpType.max)
                key = spool.tile([128, wk], _BF16, tag="key", name="key")
                u3 = u[:].rearrange("p (k x) -> p k x", x=2 * w)
                k3 = key[:].rearrange("p (k x) -> p k x", x=w)
                nc.vector.tensor_tensor(k3, u3[:, :, 0:w], u3[:, :, w : 2 * w],
                                        op=mybir.AluOpType.max)
                nc.vector.tensor_reduce(zview(zbase[gi], k), k3,
                                        axis=mybir.AxisListType.X,
                                        op=mybir.AluOpType.min)

            # split output DMA: the bulk fires while the last groups compute
            if ztail > 0:
                nc.sync.dma_start(out_d.ap()[:, 0:ztail], zmin_a[:, :])
            nc.sync.dma_start(out_d.ap()[:, ztail:nslot], zmin_b[:, :])

    nc.compile()
    return nc


def _get_nc(sched):
    if sched not in _NC_CACHE:
        _NC_CACHE[sched] = _build_nc(sched)
    return _NC_CACHE[sched]


def _planes64(vertices, faces):
    """Full-precision planes on global basis (j, i, 1): [B, 4, 3, F] f64."""
    v64 = vertices.astype(np.float64)
    fidx = np.asarray(faces).astype(np.int64).reshape(-1)
    fv = v64[:, fidx, :].reshape(_B, _F, 3, 3)
    x0, y0, z0 = fv[:, :, 0, 0], fv[:, :, 0, 1], fv[:, :, 0, 2]
    x1, y1, z1 = fv[:, :, 1, 0], fv[:, :, 1, 1], fv[:, :, 1, 2]
    x2, y2, z2 = fv[:, :, 2, 0], fv[:, :, 2, 1], fv[:, :, 2, 2]

    # area exactly as the reference computes it (float32 ops)
    v32 = vertices.astype(np.float32)
    fv32 = v32[:, fidx, :].reshape(_B, _F, 3, 3)
    xa, ya = fv32[:, :, 0, 0], fv32[:, :, 0, 1]
    xb, yb = fv32[:, :, 1, 0], fv32[:, :, 1, 1]
    xc, yc = fv32[:, :, 2, 0], fv32[:, :, 2, 1]
    area32 = (xb - xa) * (yc - ya) - (yb - ya) * (xc - xa)
    s = np.sign(area32).astype(np.float64)
    valid = np.abs(area32) > 1e-12

    A0 = -(y2 - y1); B0 = x2 - x1; C0 = (y2 - y1) * x1 - (x2 - x1) * y1
    A1 = -(y0 - y2); B1 = x0 - x2; C1 = (y0 - y2) * x2 - (x0 - x2) * y2
    A2 = -(y1 - y0); B2 = x1 - x0; C2 = (y1 - y0) * x0 - (x1 - x0) * y0

    area64 = np.where(valid, area32.astype(np.float64), 1.0)
    Aw = (z0 * A0 + z1 * A1 + z2 * A2) / area64
    Bw = (z0 * B0 + z1 * B1 + z2 * B2) / area64
    Cw = (z0 * C0 + z1 * C1 + z2 * C2) / area64

    planes = np.zeros((_B, 4, 3, _F), np.float64)
    raw = [
        (-_S * s * A0, -_S * s * B0, _OFF - _S * s * C0),
        (-_S * s * A1, -_S * s * B1, _OFF - _S * s * C1),
        (-_S * s * A2, -_S * s * B2, _OFF - _S * s * C2),
        (Aw, Bw, Cw),
    ]
    for k, (a, b, c) in enumerate(raw):
        a = np.where(valid, a, 0.0)
        b = np.where(valid, b, 0.0)
        c = np.where(valid, c, _BIGC)
        # basis change px = 5j + 2.5, py = 5i + 2.5 -> (j, i, 1)
        planes[:, k, 0] = 5.0 * a
        planes[:, k, 1] = 5.0 * b
        planes[:, k, 2] = 2.5 * a + 2.5 * b + c

    xsmin = fv[..., 0].min(2); xsmax = fv[..., 0].max(2)
    ysmin = fv[..., 1].min(2); ysmax = fv[..., 1].max(2)
    zmin_tri = fv[..., 2].min(2)
    return planes, valid, xsmin, xsmax, ysmin, ysmax, zmin_tri


def _split3(c64):
    hi = c64.astype(_BF16_NP).astype(np.float64)
    mid = (c64 - hi).astype(_BF16_NP).astype(np.float64)
    lo = (c64 - hi - mid).astype(_BF16_NP)
    return hi.astype(_BF16_NP), mid.astype(_BF16_NP), lo


def _corners(j0, i0, nj, ni):
    return np.array([[j0, i0, 1], [j0 + nj - 1, i0, 1],
                     [j0, i0 + ni - 1, 1], [j0 + nj - 1, i0 + ni - 1, 1]],
                    np.float64)


def _prepare(vertices, faces):
    planes, valid, xsmin, xsmax, ysmin, ysmax, zmin_tri = _planes64(vertices, faces)
    ntj = _W // _TJ
    ssj, ssi = _TJ // _SUBJ, _TI // _SUBI

    all_items = []
    for b in range(_B):
        P = planes[b]
        for t in range(_NTILE):
            tj, ti = t % ntj, t // ntj
            j0, i0 = tj * _TJ, ti * _TI
            xlo, xhi = 5 * j0 + 2.5, 5 * (j0 + _TJ - 1) + 2.5
            ylo, yhi = 5 * i0 + 2.5, 5 * (i0 + _TI - 1) + 2.5
            cand = np.where(valid[b] & (xsmax[b] >= xlo) & (xsmin[b] <= xhi)
                            & (ysmax[b] >= ylo) & (ysmin[b] <= yhi))[0]
            if len(cand) == 0:
                continue
            # exact subtile prune: keep a candidate iff it can win somewhere
            keep_any = np.zeros(len(cand), bool)
            Pk = [P[k][:, cand] for k in range(4)]
            zt = zmin_tri[b][cand]
            for sj in range(_SUBJ):
                for si in range(_SUBI):
                    C = _corners(j0 + sj * ssj, i0 + si * ssi, ssj, ssi)
                    alive = np.ones(len(cand), bool)
                    for k in range(3):
                        Pc = C @ Pk[k]
                        alive &= ~((Pc >= _OFF + _S * _REJ_MARGIN).all(axis=0))
                    idx = np.where(alive)[0]
                    if len(idx) == 0:
                        continue
                    Wc = C @ Pk[3][:, idx]
                    zlo = np.maximum(Wc.min(0), zt[idx])
                    covers = np.ones(len(idx), bool)
                    for k in range(3):
                        Pc = C @ Pk[k][:, idx]
                        covers &= (Pc <= _OFF - _S * _COVER_MARGIN).all(axis=0)
                    bound = (Wc.max(0)[covers].min() + _BOUND_MARGIN
                             ) if covers.any() else np.inf
                    keep_any[idx[zlo <= bound]] = True
            kept = cand[keep_any]
            if len(kept) == 0:
                continue
            Cf = _corners(j0, i0, _TJ, _TI)
            Wf = Cf @ P[3][:, kept]
            zlo_f = np.maximum(Wf.min(0), zmin_tri[b][kept])
            order = kept[np.argsort(zlo_f)]
            for c0 in range(0, len(order), _WMAX):
                all_items.append((b, t, order[c0 : c0 + _WMAX]))

    if not all_items:
        return None, None, None
    all_items.sort(key=lambda it: -len(it[2]))
    core_items = [[] for _ in range(8)]
    for r, it in enumerate(all_items):
        core_items[r % 8].append(it)

    nslot = max(len(ci) for ci in core_items)
    rawcaps = []
    for s in range(nslot):
        m = max((len(ci[s][2]) if s < len(ci) else 0) for ci in core_items)
        rawcaps.append(max(_GRAN, ((m + _GRAN - 1) // _GRAN) * _GRAN))

    if nslot % 2:
        rawcaps.append(_GRAN)
        nslot += 1
    groups = []
    s = 0
    while s < nslot:
        w = rawcaps[s]
        k = min(max(1, 512 // (2 * w)), nslot - s)
        if k > 1 and k % 2:
            k -= 1  # even k: 2x perf mode needs even dst elem count
        groups.append((w, k, s))
        s += k
    # processing order: smallest group first (its matmul+copy chain gates
    # the first DVE op), then descending size.
    order = sorted(groups, key=lambda g: g[0] * g[1])
    sched = tuple([order[0]] + sorted(order[1:], key=lambda g: -g[0] * g[1]))
    xt, strip1, colbase, pix = _region_layout(sched)

    in_maps = []
    jl = (np.arange(128) % _TJ).astype(np.float32)
    il = (np.arange(128) // _TJ).astype(np.float32)
    pix1 = np.stack([jl, il, np.ones(128, np.float32)])
    pix9 = np.vstack([pix1, pix1, pix1]).astype(_BF16_NP)
    for c in range(8):
        items = core_items[c]
        stage = np.zeros((3, xt), np.float64)
        stage[2, :] = _BIGC
        for gi, (w, k, s0) in enumerate(sched):
            og = colbase[gi]
            for q in range(k):
                s = s0 + q
                if s < len(items):
                    b, t, idx = items[s]
                    n = len(idx)
                    tj, ti = t % ntj, t // ntj
                    j0, i0 = tj * _TJ, ti * _TI
                    Pl = np.empty((4, 3, n))
                    for kk in range(4):
                        a = planes[b, kk, 0, idx]
                        b_ = planes[b, kk, 1, idx]
                        cc = planes[b, kk, 2, idx] + a * j0 + b_ * i0
                        Pl[kk] = np.stack([a, b_, cc])
                    cA = og + 2 * w * q
                    cB = og + 2 * w * k + 2 * w * q
                    stage[:, cA : cA + n] = Pl[0]          # P0
                    stage[:, cA + w : cA + w + n] = Pl[3]  # W
                    stage[:, cB : cB + n] = Pl[1]          # P1
                    stage[:, cB + w : cB + w + n] = Pl[2]  # P2
        hi, mid, lo = _split3(stage)
        data = np.concatenate([hi, mid, lo], axis=0)
        data[0:9, pix : pix + 128] = pix9
        in_maps.append({"data": data})
    return sched, in_maps, core_items


def kernel(vertices, faces):
    vertices = np.asarray(vertices)
    faces = np.asarray(faces)
    sched, in_maps, core_items = _prepare(vertices, faces)
    if sched is None:
        return np.full((_B, _H, _W), _CLAMP, np.float32)

    nc = _get_nc(sched)
    kw = dict(PROFILE.get("run_kwargs", {}))
    res = run_bass_kernel_spmd(nc, in_maps, list(range(8)), **kw)
    PROFILE["last_result"] = res

    # device zmin columns are in processing order; map rank -> column
    rank2col = {}
    z0 = 0
    for w, k, s0 in sched:
        for q in range(k):
            rank2col[s0 + q] = z0 + q
        z0 += k

    ntj = _W // _TJ
    out = np.full((_B, _H, _W), _CLAMP, np.float32)
    for c in range(8):
        zraw = np.asarray(res.results[c]["out"]).astype(np.float32)  # [128, nslot]
        z = np.empty_like(zraw)
        for s, col in rank2col.items():
            z[:, s] = zraw[:, col]
        for s, (b, t, idx) in enumerate(core_items[c]):
            tj, ti = t % ntj, t // ntj
            j0, i0 = tj * _TJ, ti * _TI
            blk = z[:, s].reshape(_TI, _TJ)
            out[b, i0 : i0 + _TI, j0 : j0 + _TJ] = np.minimum(
                out[b, i0 : i0 + _TI, j0 : j0 + _TJ], blk)
    return out


# revision 58
# speedup vs baseline: 1.0350x; 1.0350x over previous
"""Depth rasterization (MANO hand z-buffer @ 640x640 -> bilinear 128x128).

Key identities exploited:
  * jax.image.resize(640->128, linear, antialias=False) samples input coords
    5*j + 2.0 exactly -> output[i, j] == raster[5i+2, 5j+2]. Only the 128x128
    decimated pixel grid (centers x = 5j+2.5, y = 5i+2.5) is rasterized: a
    25x reduction vs the reference's 640x640 raster.
  * Edge functions and barycentric depth are affine in pixel coords, so each
    triangle yields four planes over the local basis (jl, il, 1) of its tile
    (tile origin folded into the constant term on host):
      P_k = OFF - S * sign(area) * e_k     (k = 0,1,2 penalty planes)
      W   = (e0*z0 + e1*z1 + e2*z2) / area (depth plane)
    key(p, f) = max(P0, P1, P2, W) equals the interpolated depth when p is
    inside triangle f and is >= OFF (>> the 100 clamp) outside; the z-buffer
    is zbuf(p) = min(100, min_f key(p, f)).
  * Plane evaluation is a K=9 bf16 matmul (coefficients split into 3 bf16
    limbs; the (jl, il, 1) basis is exact in bf16, giving fp32-grade accuracy
    at bf16 PE speed). Per slot of width w the PE writes [P0|W] (bank A,
    quadrant ra) and [P1|P2] (bank B, quadrant rb); ScalarE pulls bank A to
    SBUF (DVE reads at most one PSUM operand), DVE takes u = max(ta, pb)
    (bf16 out), pair-max within each slot (3D APs, one instr per group), and
    min-reduces over candidates (axis=X over [128, k, w]).
  * Per 16x8-pixel tile, candidates are bbox-filtered, then pruned exactly on
    an 8x8 subtile grid: a candidate is dropped if in every subtile it either
    misses the subtile entirely (some edge has all 4 corners outside by a
    margin safely above the reference's own fp32 edge-function noise) or its
    minimum possible depth exceeds the subtile's best fully-covering
    candidate's maximum depth. Exact vs the reference for any input.
  * Only the 9 coefficient rows actually read by the PE are shipped (HBM
    [9, cols]); the pixel basis is one shared [9, 128] tensor replicated into
    the 4 PE quadrants on device.

Sharding: 8 cores; (batch, tile) work items rank-balanced across all cores.
"""

import numpy as np
import ml_dtypes

import concourse.bacc as bacc
import concourse.mybir as mybir
import concourse.tile as tile
from concourse.bass_utils import run_bass_kernel_spmd

_B, _V, _F = 4, 778, 1538
_H = _W = 128
_TJ, _TI = 16, 8   # tile size in output pixels (x, y)
_NTILE = (_H // _TI) * (_W // _TJ)  # 128 tiles per batch image
_SUBJ, _SUBI = 8, 8  # subtile grid for host pruning
_WMAX = 256        # max slot width
_GRAN = 2          # slot width granularity (even: keeps 4B-aligned pair APs)
_OFF = 1000.0      # penalty-plane offset (>> 100 clamp)
_S = 1.0e9         # penalty scale
_BIGC = 1.0e7      # plane constant for padding/invalid
_CLAMP = 100.0
_COVER_MARGIN = 0.3    # e*s margin (e-units) for the full-cover test
_REJ_MARGIN = 0.3      # e*s margin (e-units) for exact edge rejection
_BOUND_MARGIN = 1e-3   # depth margin for the prune bound

_F32 = mybir.dt.float32
_BF16 = mybir.dt.bfloat16
_BF16_NP = ml_dtypes.bfloat16

_NC_CACHE = {}
PROFILE = {}


def _parity(sched):
    """Quadrant-pair per sched position: the first three groups use the even
    pair (their data rides the first two DMA strips, so the pipeline start
    never waits on the odd strips); the rest alternate starting odd."""
    return [0 if gi < 3 else ((gi - 3) % 2 == 0 and 1 or 0)
            for gi in range(len(sched))]


def _region_layout(sched):
    """sched: ((w, k, s0), ...) in device processing order; position parity
    picks the quadrant pair. Column layout of the [18, XT] HBM data tensor:
    [first-even | pixE | rest-even | first-odd | pixO | rest-odd]. Rows 0-8
    carry comp-A limbs (quadrants 0/64), rows 9-17 comp-B (32/96)."""
    par = _parity(sched)
    colbase = [0] * len(sched)
    # groups 0-1: comp-B lives in the same row strip as comp-A (both matmuls
    # on quadrant 0, back to back — the scheduler batches by quadrant and
    # would otherwise defer their pb behind later groups' pa matmuls), and
    # the whole early chain depends only on the first DMA strip.
    cbB = {0: 2 * sched[0][0] * sched[0][1]}
    o = cbB[0] + 2 * sched[0][0] * sched[0][1]
    pixe = o
    o += 128
    for gi in range(1, len(sched)):
        if par[gi] == 0:
            colbase[gi] = o
            o += 2 * sched[gi][0] * sched[gi][1]
    xe = o
    pixo = None
    for gi in range(1, len(sched)):
        if par[gi] == 1:
            if pixo is None:
                pixo = o
                o += 128
            colbase[gi] = o
            o += 2 * sched[gi][0] * sched[gi][1]
    if pixo is None:
        pixo = pixe
    return o, xe, colbase, pixe, pixo, cbB


def _build_nc(sched):
    """sched: ((w, k, s0), ...) slots padded to width w, k per group, with
    2*w*k <= 512 (one PSUM bank per component pair)."""
    nslot = sum(k for _, k, _ in sched)
    xt, xe, colbase, pixe, pixo, cbB = _region_layout(sched)
    par = _parity(sched)
    nc = bacc.Bacc("TRN2", target_bir_lowering=False, debug=False, num_devices=8)
    data_d = nc.dram_tensor("data", [18, xt], _BF16, kind="ExternalInput")
    out_d = nc.dram_tensor("out", [128, nslot], _BF16, kind="ExternalOutput")

    with tile.TileContext(nc) as tc:
        with (
            tc.tile_pool(name="const", bufs=1) as cpool,
            tc.tile_pool(name="scr", bufs=6) as spool,
            tc.tile_pool(name="ps", bufs=4, space="PSUM") as ppool,
        ):
            ct = cpool.tile([128, xt], _BF16, name="ct")
            # 4 quadrant-strip DMAs (descriptor writes are ~0.7us serial on
            # the Sync sequencer; the transfers themselves are tiny).
            nc.sync.dma_start(ct[0:9, 0:xe], data_d.ap()[0:9, 0:xe])
            nc.sync.dma_start(ct[32:41, 0:xe], data_d.ap()[9:18, 0:xe])
            if xe < xt:
                nc.sync.dma_start(ct[64:73, xe:xt], data_d.ap()[0:9, xe:xt])
                nc.sync.dma_start(ct[96:105, xe:xt], data_d.ap()[9:18, xe:xt])
            # zmin columns in processing order (host maps back), split into
            # two tiles so the bulk output DMA's dependency resolves before
            # the last two (smallest) groups finish.
            z0 = 0
            zbase = []
            for w, k, s0 in sched:
                zbase.append(z0)
                z0 += k
            ztail = zbase[-1] if len(sched) > 1 else 0
            zmin_a = cpool.tile([128, max(ztail, 1)], _BF16, name="zmina")
            zmin_b = cpool.tile([128, nslot - ztail], _BF16, name="zminb")

            def zview(col0, k):
                if col0 >= ztail:
                    return zmin_b[:, col0 - ztail : col0 - ztail + k]
                return zmin_a[:, col0 : col0 + k]

            # dual-copy the largest groups: ScalarE pulls BOTH psum banks to
            # bf16 SBUF so their wide-max runs at DVE 2x (all-2B operands).
            # bf16(max) == max(bf16) exactly, so accuracy is unchanged.
            # Balance: each dual group adds ~0.61us scalar, saves ~0.33us DVE.
            dual = set()  # dual-copy measured as a wash: scalar serialization
                          # per group eats the DVE 2x win

            for gi, (w, k, s0) in enumerate(sched):
                wk = w * k
                ra, rb = (0, 32) if par[gi] == 0 else (64, 96)
                px = pixe if par[gi] == 0 else pixo
                og = colbase[gi]
                pa = ppool.tile([128, 2 * wk], _F32, tag="pa", name="pa")
                pb = ppool.tile([128, 2 * wk], _F32, tag="pb", name="pb")
                nc.tensor.matmul(pa[:, :], ct[ra : ra + 9, px : px + 128],
                                 ct[ra : ra + 9, og : og + 2 * wk],
                                 start=True, stop=True, tile_position=(ra, 0))
                if gi in cbB:
                    nc.tensor.matmul(pb[:, :], ct[ra : ra + 9, px : px + 128],
                                     ct[ra : ra + 9, cbB[gi] : cbB[gi] + 2 * wk],
                                     start=True, stop=True,
                                     tile_position=(ra, 0))
                else:
                    nc.tensor.matmul(pb[:, :], ct[rb : rb + 9, px : px + 128],
                                     ct[rb : rb + 9, og : og + 2 * wk],
                                     start=True, stop=True,
                                     tile_position=(rb, 0))
                # ScalarE pulls comp-A to SBUF (DVE reads max one PSUM operand)
                u = spool.tile([128, 2 * wk], _BF16, tag="u", name="u")
                if gi in dual:
                    ta = spool.tile([128, 2 * wk], _BF16, tag="ta", name="ta")
                    tb = spool.tile([128, 2 * wk], _BF16, tag="tb", name="tb")
                    nc.scalar.copy(ta[:, :], pa[:, :])
                    nc.scalar.copy(tb[:, :], pb[:, :])
                    nc.vector.tensor_tensor(u[:, :], ta[:, :], tb[:, :],
                                            op=mybir.AluOpType.max)
                else:
                    ta = spool.tile([128, 2 * wk], _F32, tag="ta", name="ta")
                    nc.scalar.copy(ta[:, :], pa[:, :])
                    nc.vector.tensor_tensor(u[:, :], ta[:, :], pb[:, :],
                                            op=mybir.AluOpType.max)
                key = spool.tile([128, wk], _BF16, tag="key", name="key")
                u3 = u[:].rearrange("p (k x) -> p k x", x=2 * w)
                k3 = key[:].rearrange("p (k x) -> p k x", x=w)
                nc.vector.tensor_tensor(k3, u3[:, :, 0:w], u3[:, :, w : 2 * w],
                                        op=mybir.AluOpType.max)
                nc.vector.tensor_reduce(zview(zbase[gi], k), k3,
                                        axis=mybir.AxisListType.X,
                                        op=mybir.AluOpType.min)

            # split output DMA: the bulk fires while the last groups compute
            if ztail > 0:
                nc.sync.dma_start(out_d.ap()[:, 0:ztail], zmin_a[:, :])
            nc.sync.dma_start(out_d.ap()[:, ztail:nslot], zmin_b[:, :])

    nc.compile()
    return nc


def _get_nc(sched):
    if sched not in _NC_CACHE:
        _NC_CACHE[sched] = _build_nc(sched)
    return _NC_CACHE[sched]


def _planes64(vertices, faces):
    """Full-precision planes on global basis (j, i, 1): [B, 4, 3, F] f64."""
    v64 = vertices.astype(np.float64)
    fidx = np.asarray(faces).astype(np.int64).reshape(-1)
    fv = v64[:, fidx, :].reshape(_B, _F, 3, 3)
    x0, y0, z0 = fv[:, :, 0, 0], fv[:, :, 0, 1], fv[:, :, 0, 2]
    x1, y1, z1 = fv[:, :, 1, 0], fv[:, :, 1, 1], fv[:, :, 1, 2]
    x2, y2, z2 = fv[:, :, 2, 0], fv[:, :, 2, 1], fv[:, :, 2, 2]

    # area exactly as the reference computes it (float32 ops)
    v32 = vertices.astype(np.float32)
    fv32 = v32[:, fidx, :].reshape(_B, _F, 3, 3)
    xa, ya = fv32[:, :, 0, 0], fv32[:, :, 0, 1]
    xb, yb = fv32[:, :, 1, 0], fv32[:, :, 1, 1]
    xc, yc = fv32[:, :, 2, 0], fv32[:, :, 2, 1]
    area32 = (xb - xa) * (yc - ya) - (yb - ya) * (xc - xa)
    s = np.sign(area32).astype(np.float64)
    valid = np.abs(area32) > 1e-12

    A0 = -(y2 - y1); B0 = x2 - x1; C0 = (y2 - y1) * x1 - (x2 - x1) * y1
    A1 = -(y0 - y2); B1 = x0 - x2; C1 = (y0 - y2) * x2 - (x0 - x2) * y2
    A2 = -(y1 - y0); B2 = x1 - x0; C2 = (y1 - y0) * x0 - (x1 - x0) * y0

    area64 = np.where(valid, area32.astype(np.float64), 1.0)
    Aw = (z0 * A0 + z1 * A1 + z2 * A2) / area64
    Bw = (z0 * B0 + z1 * B1 + z2 * B2) / area64
    Cw = (z0 * C0 + z1 * C1 + z2 * C2) / area64

    planes = np.zeros((_B, 4, 3, _F), np.float64)
    raw = [
        (-_S * s * A0, -_S * s * B0, _OFF - _S * s * C0),
        (-_S * s * A1, -_S * s * B1, _OFF - _S * s * C1),
        (-_S * s * A2, -_S * s * B2, _OFF - _S * s * C2),
        (Aw, Bw, Cw),
    ]
    for k, (a, b, c) in enumerate(raw):
        a = np.where(valid, a, 0.0)
        b = np.where(valid, b, 0.0)
        c = np.where(valid, c, _BIGC)
        # basis change px = 5j + 2.5, py = 5i + 2.5 -> (j, i, 1)
        planes[:, k, 0] = 5.0 * a
        planes[:, k, 1] = 5.0 * b
        planes[:, k, 2] = 2.5 * a + 2.5 * b + c

    xsmin = fv[..., 0].min(2); xsmax = fv[..., 0].max(2)
    ysmin = fv[..., 1].min(2); ysmax = fv[..., 1].max(2)
    zmin_tri = fv[..., 2].min(2)
    return planes, valid, xsmin, xsmax, ysmin, ysmax, zmin_tri


def _split3(c64):
    hi = c64.astype(_BF16_NP).astype(np.float64)
    mid = (c64 - hi).astype(_BF16_NP).astype(np.float64)
    lo = (c64 - hi - mid).astype(_BF16_NP)
    return hi.astype(_BF16_NP), mid.astype(_BF16_NP), lo


def _corners(j0, i0, nj, ni):
    return np.array([[j0, i0, 1], [j0 + nj - 1, i0, 1],
                     [j0, i0 + ni - 1, 1], [j0 + nj - 1, i0 + ni - 1, 1]],
                    np.float64)


def _prepare(vertices, faces):
    planes, valid, xsmin, xsmax, ysmin, ysmax, zmin_tri = _planes64(vertices, faces)
    ntj = _W // _TJ
    ssj, ssi = _TJ // _SUBJ, _TI // _SUBI

    all_items = []
    for b in range(_B):
        P = planes[b]
        for t in range(_NTILE):
            tj, ti = t % ntj, t // ntj
            j0, i0 = tj * _TJ, ti * _TI
            xlo, xhi = 5 * j0 + 2.5, 5 * (j0 + _TJ - 1) + 2.5
            ylo, yhi = 5 * i0 + 2.5, 5 * (i0 + _TI - 1) + 2.5
            cand = np.where(valid[b] & (xsmax[b] >= xlo) & (xsmin[b] <= xhi)
                            & (ysmax[b] >= ylo) & (ysmin[b] <= yhi))[0]
            if len(cand) == 0:
                continue
            # exact subtile prune: keep a candidate iff it can win somewhere
            keep_any = np.zeros(len(cand), bool)
            Pk = [P[k][:, cand] for k in range(4)]
            zt = zmin_tri[b][cand]
            for sj in range(_SUBJ):
                for si in range(_SUBI):
                    C = _corners(j0 + sj * ssj, i0 + si * ssi, ssj, ssi)
                    alive = np.ones(len(cand), bool)
                    for k in range(3):
                        Pc = C @ Pk[k]
                        alive &= ~((Pc >= _OFF + _S * _REJ_MARGIN).all(axis=0))
                    idx = np.where(alive)[0]
                    if len(idx) == 0:
                        continue
                    Wc = C @ Pk[3][:, idx]
                    zlo = np.maximum(Wc.min(0), zt[idx])
                    covers = np.ones(len(idx), bool)
                    for k in range(3):
                        Pc = C @ Pk[k][:, idx]
                        covers &= (Pc <= _OFF - _S * _COVER_MARGIN).all(axis=0)
                    bound = (Wc.max(0)[covers].min() + _BOUND_MARGIN
                             ) if covers.any() else np.inf
                    keep_any[idx[zlo <= bound]] = True
            kept = cand[keep_any]
            if len(kept) == 0:
                continue
            Cf = _corners(j0, i0, _TJ, _TI)
            Wf = Cf @ P[3][:, kept]
            zlo_f = np.maximum(Wf.min(0), zmin_tri[b][kept])
            order = kept[np.argsort(zlo_f)]
            for c0 in range(0, len(order), _WMAX):
                all_items.append((b, t, order[c0 : c0 + _WMAX]))

    if not all_items:
        return None, None, None
    all_items.sort(key=lambda it: -len(it[2]))
    core_items = [[] for _ in range(8)]
    for r, it in enumerate(all_items):
        core_items[r % 8].append(it)

    nslot = max(len(ci) for ci in core_items)
    rawcaps = []
    for s in range(nslot):
        m = max((len(ci[s][2]) if s < len(ci) else 0) for ci in core_items)
        rawcaps.append(max(_GRAN, ((m + _GRAN - 1) // _GRAN) * _GRAN))

    if nslot % 2:
        rawcaps.append(_GRAN)
        nslot += 1
    groups = []
    s = 0
    while s < nslot:
        w = rawcaps[s]
        k = min(max(1, 512 // (2 * w)), nslot - s)
        if k > 1 and k % 2:
            k -= 1  # even k: 2x perf mode needs even dst elem count
        groups.append((w, k, s))
        s += k
    # processing order: smallest group first (its matmul+copy chain gates
    # the first DVE op), then descending size.
    order = sorted(groups, key=lambda g: g[0] * g[1])
    sched = tuple([order[0]] + sorted(order[1:], key=lambda g: -g[0] * g[1]))
    xt, xe, colbase, pixe, pixo, cbB = _region_layout(sched)

    in_maps = []
    jl = (np.arange(128) % _TJ).astype(np.float32)
    il = (np.arange(128) // _TJ).astype(np.float32)
    pix1 = np.stack([jl, il, np.ones(128, np.float32)])
    pix9 = np.vstack([pix1, pix1, pix1]).astype(_BF16_NP)
    for c in range(8):
        items = core_items[c]
        stageA = np.zeros((3, xt), np.float64)
        stageB = np.zeros((3, xt), np.float64)
        stageA[2, :] = _BIGC
        stageB[2, :] = _BIGC
        for gi, (w, k, s0) in enumerate(sched):
            og = colbase[gi]
            for q in range(k):
                s = s0 + q
                if s < len(items):
                    b, t, idx = items[s]
                    n = len(idx)
                    tj, ti = t % ntj, t // ntj
                    j0, i0 = tj * _TJ, ti * _TI
                    Pl = np.empty((4, 3, n))
                    for kk in range(4):
                        a = planes[b, kk, 0, idx]
                        b_ = planes[b, kk, 1, idx]
                        cc = planes[b, kk, 2, idx] + a * j0 + b_ * i0
                        Pl[kk] = np.stack([a, b_, cc])
                    cA = og + 2 * w * q
                    stageA[:, cA : cA + n] = Pl[0]          # P0
                    stageA[:, cA + w : cA + w + n] = Pl[3]  # W
                    sB, cB = (stageA, cbB[gi] + 2 * w * q) if gi in cbB else (stageB, cA)
                    sB[:, cB : cB + n] = Pl[1]          # P1
                    sB[:, cB + w : cB + w + n] = Pl[2]  # P2
        data = np.zeros((18, xt), _BF16_NP)
        for rbase, stage in ((0, stageA), (9, stageB)):
            hi, mid, lo = _split3(stage)
            data[rbase : rbase + 9] = np.concatenate([hi, mid, lo], axis=0)
        for px in {pixe, pixo}:
            data[0:9, px : px + 128] = pix9
            data[9:18, px : px + 128] = pix9
        in_maps.append({"data": data})
    return sched, in_maps, core_items


def kernel(vertices, faces):
    vertices = np.asarray(vertices)
    faces = np.asarray(faces)
    sched, in_maps, core_items = _prepare(vertices, faces)
    if sched is None:
        return np.full((_B, _H, _W), _CLAMP, np.float32)

    nc = _get_nc(sched)
    kw = dict(PROFILE.get("run_kwargs", {}))
    res = run_bass_kernel_spmd(nc, in_maps, list(range(8)), **kw)
    PROFILE["last_result"] = res

    # device zmin columns are in processing order; map rank -> column
    rank2col = {}
    z0 = 0
    for w, k, s0 in sched:
        for q in range(k):
            rank2col[s0 + q] = z0 + q
        z0 += k

    ntj = _W // _TJ
    out = np.full((_B, _H, _W), _CLAMP, np.float32)
    for c in range(8):
        zraw = np.asarray(res.results[c]["out"]).astype(np.float32)  # [128, nslot]
        z = np.empty_like(zraw)
        for s, col in rank2col.items():
            z[:, s] = zraw[:, col]
        for s, (b, t, idx) in enumerate(core_items[c]):
            tj, ti = t % ntj, t // ntj
            j0, i0 = tj * _TJ, ti * _TI
            blk = z[:, s].reshape(_TI, _TJ)
            out[b, i0 : i0 + _TI, j0 : j0 + _TJ] = np.minimum(
                out[b, i0 : i0 + _TI, j0 : j0 + _TJ], blk)
    return out


# revision 59
# speedup vs baseline: 1.0456x; 1.0103x over previous
"""Depth rasterization (MANO hand z-buffer @ 640x640 -> bilinear 128x128).

Key identities exploited:
  * jax.image.resize(640->128, linear, antialias=False) samples input coords
    5*j + 2.0 exactly -> output[i, j] == raster[5i+2, 5j+2]. Only the 128x128
    decimated pixel grid (centers x = 5j+2.5, y = 5i+2.5) is rasterized: a
    25x reduction vs the reference's 640x640 raster.
  * Edge functions and barycentric depth are affine in pixel coords, so each
    triangle yields four planes over the local basis (jl, il, 1) of its tile
    (tile origin folded into the constant term on host):
      P_k = OFF - S * sign(area) * e_k     (k = 0,1,2 penalty planes)
      W   = (e0*z0 + e1*z1 + e2*z2) / area (depth plane)
    key(p, f) = max(P0, P1, P2, W) equals the interpolated depth when p is
    inside triangle f and is >= OFF (>> the 100 clamp) outside; the z-buffer
    is zbuf(p) = min(100, min_f key(p, f)).
  * Plane evaluation is a K=9 bf16 matmul (coefficients split into 3 bf16
    limbs; the (jl, il, 1) basis is exact in bf16, giving fp32-grade accuracy
    at bf16 PE speed). Per slot of width w the PE writes [P0|W] (bank A,
    quadrant ra) and [P1|P2] (bank B, quadrant rb); ScalarE pulls bank A to
    SBUF (DVE reads at most one PSUM operand), DVE takes u = max(ta, pb)
    (bf16 out), pair-max within each slot (3D APs, one instr per group), and
    min-reduces over candidates (axis=X over [128, k, w]).
  * Per 16x8-pixel tile, candidates are bbox-filtered, then pruned exactly on
    an 8x8 subtile grid: a candidate is dropped if in every subtile it either
    misses the subtile entirely (some edge has all 4 corners outside by a
    margin safely above the reference's own fp32 edge-function noise) or its
    minimum possible depth exceeds the subtile's best fully-covering
    candidate's maximum depth. Exact vs the reference for any input.
  * Only the 9 coefficient rows actually read by the PE are shipped (HBM
    [9, cols]); the pixel basis is one shared [9, 128] tensor replicated into
    the 4 PE quadrants on device.

Sharding: 8 cores; (batch, tile) work items rank-balanced across all cores.
"""

import numpy as np
import ml_dtypes

import concourse.bacc as bacc
import concourse.mybir as mybir
import concourse.tile as tile
from concourse.bass_utils import run_bass_kernel_spmd

_B, _V, _F = 4, 778, 1538
_H = _W = 128
_TJ, _TI = 16, 8   # tile size in output pixels (x, y)
_NTILE = (_H // _TI) * (_W // _TJ)  # 128 tiles per batch image
_SUBJ, _SUBI = 8, 8  # subtile grid for host pruning
_WMAX = 256        # max slot width
_GRAN = 2          # slot width granularity (even: keeps 4B-aligned pair APs)
_OFF = 1000.0      # penalty-plane offset (>> 100 clamp)
_S = 1.0e9         # penalty scale
_BIGC = 1.0e7      # plane constant for padding/invalid
_CLAMP = 100.0
_COVER_MARGIN = 0.3    # e*s margin (e-units) for the full-cover test
_REJ_MARGIN = 0.3      # e*s margin (e-units) for exact edge rejection
_BOUND_MARGIN = 1e-3   # depth margin for the prune bound

_F32 = mybir.dt.float32
_BF16 = mybir.dt.bfloat16
_BF16_NP = ml_dtypes.bfloat16

_NC_CACHE = {}
PROFILE = {}


def _parity(sched):
    """Quadrant-pair per sched position: the first three groups use the even
    pair (their data rides the first two DMA strips, so the pipeline start
    never waits on the odd strips); the rest alternate starting odd."""
    return [0 if gi < 3 else ((gi - 3) % 2 == 0 and 1 or 0)
            for gi in range(len(sched))]


def _region_layout(sched):
    """sched: ((w, k, s0), ...) in device processing order; position parity
    picks the quadrant pair. Column layout of the [18, XT] HBM data tensor:
    [first-even | pixE | rest-even | first-odd | pixO | rest-odd]. Rows 0-8
    carry comp-A limbs (quadrants 0/64), rows 9-17 comp-B (32/96)."""
    par = _parity(sched)
    colbase = [0] * len(sched)
    # groups 0-1: comp-B lives in the same row strip as comp-A (both matmuls
    # on quadrant 0, back to back — the scheduler batches by quadrant and
    # would otherwise defer their pb behind later groups' pa matmuls), and
    # the whole early chain depends only on the first DMA strip.
    cbB = {0: 2 * sched[0][0] * sched[0][1]}
    o = cbB[0] + 2 * sched[0][0] * sched[0][1]
    pixe = o
    o += 128
    for gi in range(1, len(sched)):
        if par[gi] == 0:
            colbase[gi] = o
            o += 2 * sched[gi][0] * sched[gi][1]
    xe = o
    pixo = None
    for gi in range(1, len(sched)):
        if par[gi] == 1:
            if pixo is None:
                pixo = o
                o += 128
            colbase[gi] = o
            o += 2 * sched[gi][0] * sched[gi][1]
    if pixo is None:
        pixo = pixe
    return o, xe, colbase, pixe, pixo, cbB


def _build_nc(sched):
    """sched: ((w, k, s0), ...) slots padded to width w, k per group, with
    2*w*k <= 512 (one PSUM bank per component pair)."""
    nslot = sum(k for _, k, _ in sched)
    xt, xe, colbase, pixe, pixo, cbB = _region_layout(sched)
    par = _parity(sched)
    nc = bacc.Bacc("TRN2", target_bir_lowering=False, debug=False, num_devices=8)
    data_d = nc.dram_tensor("data", [18, xt], _BF16, kind="ExternalInput")
    out_d = nc.dram_tensor("out", [128, nslot], _BF16, kind="ExternalOutput")

    with tile.TileContext(nc) as tc:
        with (
            tc.tile_pool(name="const", bufs=1) as cpool,
            tc.tile_pool(name="scr", bufs=6) as spool,
            tc.tile_pool(name="ps", bufs=4, space="PSUM") as ppool,
        ):
            ct = cpool.tile([128, xt], _BF16, name="ct")
            # 4 quadrant-strip DMAs (descriptor writes are ~0.7us serial on
            # the Sync sequencer; the transfers themselves are tiny).
            nc.sync.dma_start(ct[0:9, 0:xe], data_d.ap()[0:9, 0:xe])
            nc.sync.dma_start(ct[32:41, 0:xe], data_d.ap()[9:18, 0:xe])
            if xe < xt:
                nc.sync.dma_start(ct[64:73, xe:xt], data_d.ap()[0:9, xe:xt])
                nc.sync.dma_start(ct[96:105, xe:xt], data_d.ap()[9:18, xe:xt])
            # zmin columns in processing order (host maps back), split into
            # two tiles so the bulk output DMA's dependency resolves before
            # the last two (smallest) groups finish.
            z0 = 0
            zbase = []
            for w, k, s0 in sched:
                zbase.append(z0)
                z0 += k
            ztail = zbase[-1] if len(sched) > 1 else 0
            zmin_a = cpool.tile([128, max(ztail, 1)], _BF16, name="zmina")
            zmin_b = cpool.tile([128, nslot - ztail], _BF16, name="zminb")

            def zview(col0, k):
                if col0 >= ztail:
                    return zmin_b[:, col0 - ztail : col0 - ztail + k]
                return zmin_a[:, col0 : col0 + k]

            # dual-copy the largest groups: ScalarE pulls BOTH psum banks to
            # bf16 SBUF so their wide-max runs at DVE 2x (all-2B operands).
            # bf16(max) == max(bf16) exactly, so accuracy is unchanged.
            # Balance: each dual group adds ~0.61us scalar, saves ~0.33us DVE.
            dual = set()  # dual-copy measured as a wash: scalar serialization
                          # per group eats the DVE 2x win

            for gi, (w, k, s0) in enumerate(sched):
                wk = w * k
                ra, rb = (0, 32) if par[gi] == 0 else (64, 96)
                px = pixe if par[gi] == 0 else pixo
                og = colbase[gi]
                pa = ppool.tile([128, 2 * wk], _F32, tag="pa", name="pa")
                pb = ppool.tile([128, 2 * wk], _F32, tag="pb", name="pb")
                nc.tensor.matmul(pa[:, :], ct[ra : ra + 9, px : px + 128],
                                 ct[ra : ra + 9, og : og + 2 * wk],
                                 start=True, stop=True, tile_position=(ra, 0))
                if gi in cbB:
                    nc.tensor.matmul(pb[:, :], ct[ra : ra + 9, px : px + 128],
                                     ct[ra : ra + 9, cbB[gi] : cbB[gi] + 2 * wk],
                                     start=True, stop=True,
                                     tile_position=(ra, 0))
                else:
                    nc.tensor.matmul(pb[:, :], ct[rb : rb + 9, px : px + 128],
                                     ct[rb : rb + 9, og : og + 2 * wk],
                                     start=True, stop=True,
                                     tile_position=(rb, 0))
                # ScalarE pulls comp-A to SBUF (DVE reads max one PSUM operand)
                u = spool.tile([128, 2 * wk], _BF16, tag="u", name="u")
                if gi in dual:
                    ta = spool.tile([128, 2 * wk], _BF16, tag="ta", name="ta")
                    tb = spool.tile([128, 2 * wk], _BF16, tag="tb", name="tb")
                    nc.scalar.copy(ta[:, :], pa[:, :])
                    nc.scalar.copy(tb[:, :], pb[:, :])
                    nc.vector.tensor_tensor(u[:, :], ta[:, :], tb[:, :],
                                            op=mybir.AluOpType.max)
                else:
                    ta = spool.tile([128, 2 * wk], _F32, tag="ta", name="ta")
                    nc.scalar.copy(ta[:, :], pa[:, :])
                    nc.vector.tensor_tensor(u[:, :], ta[:, :], pb[:, :],
                                            op=mybir.AluOpType.max)
                key = spool.tile([128, wk], _BF16, tag="key", name="key")
                u3 = u[:].rearrange("p (k x) -> p k x", x=2 * w)
                k3 = key[:].rearrange("p (k x) -> p k x", x=w)
                nc.vector.tensor_tensor(k3, u3[:, :, 0:w], u3[:, :, w : 2 * w],
                                        op=mybir.AluOpType.max)
                nc.vector.tensor_reduce(zview(zbase[gi], k), k3,
                                        axis=mybir.AxisListType.X,
                                        op=mybir.AluOpType.min)

            # split output DMA: the bulk fires while the last groups compute
            if ztail > 0:
                nc.sync.dma_start(out_d.ap()[:, 0:ztail], zmin_a[:, :])
            nc.sync.dma_start(out_d.ap()[:, ztail:nslot], zmin_b[:, :])

    nc.compile()
    return nc


def _get_nc(sched):
    if sched not in _NC_CACHE:
        _NC_CACHE[sched] = _build_nc(sched)
    return _NC_CACHE[sched]


def _planes64(vertices, faces):
    """Full-precision planes on global basis (j, i, 1): [B, 4, 3, F] f64."""
    v64 = vertices.astype(np.float64)
    fidx = np.asarray(faces).astype(np.int64).reshape(-1)
    fv = v64[:, fidx, :].reshape(_B, _F, 3, 3)
    x0, y0, z0 = fv[:, :, 0, 0], fv[:, :, 0, 1], fv[:, :, 0, 2]
    x1, y1, z1 = fv[:, :, 1, 0], fv[:, :, 1, 1], fv[:, :, 1, 2]
    x2, y2, z2 = fv[:, :, 2, 0], fv[:, :, 2, 1], fv[:, :, 2, 2]

    # area exactly as the reference computes it (float32 ops)
    v32 = vertices.astype(np.float32)
    fv32 = v32[:, fidx, :].reshape(_B, _F, 3, 3)
    xa, ya = fv32[:, :, 0, 0], fv32[:, :, 0, 1]
    xb, yb = fv32[:, :, 1, 0], fv32[:, :, 1, 1]
    xc, yc = fv32[:, :, 2, 0], fv32[:, :, 2, 1]
    area32 = (xb - xa) * (yc - ya) - (yb - ya) * (xc - xa)
    s = np.sign(area32).astype(np.float64)
    valid = np.abs(area32) > 1e-12

    A0 = -(y2 - y1); B0 = x2 - x1; C0 = (y2 - y1) * x1 - (x2 - x1) * y1
    A1 = -(y0 - y2); B1 = x0 - x2; C1 = (y0 - y2) * x2 - (x0 - x2) * y2
    A2 = -(y1 - y0); B2 = x1 - x0; C2 = (y1 - y0) * x0 - (x1 - x0) * y0

    area64 = np.where(valid, area32.astype(np.float64), 1.0)
    Aw = (z0 * A0 + z1 * A1 + z2 * A2) / area64
    Bw = (z0 * B0 + z1 * B1 + z2 * B2) / area64
    Cw = (z0 * C0 + z1 * C1 + z2 * C2) / area64

    planes = np.zeros((_B, 4, 3, _F), np.float64)
    raw = [
        (-_S * s * A0, -_S * s * B0, _OFF - _S * s * C0),
        (-_S * s * A1, -_S * s * B1, _OFF - _S * s * C1),
        (-_S * s * A2, -_S * s * B2, _OFF - _S * s * C2),
        (Aw, Bw, Cw),
    ]
    for k, (a, b, c) in enumerate(raw):
        a = np.where(valid, a, 0.0)
        b = np.where(valid, b, 0.0)
        c = np.where(valid, c, _BIGC)
        # basis change px = 5j + 2.5, py = 5i + 2.5 -> (j, i, 1)
        planes[:, k, 0] = 5.0 * a
        planes[:, k, 1] = 5.0 * b
        planes[:, k, 2] = 2.5 * a + 2.5 * b + c

    xsmin = fv[..., 0].min(2); xsmax = fv[..., 0].max(2)
    ysmin = fv[..., 1].min(2); ysmax = fv[..., 1].max(2)
    zmin_tri = fv[..., 2].min(2)
    return planes, valid, xsmin, xsmax, ysmin, ysmax, zmin_tri


def _split3(c64):
    hi = c64.astype(_BF16_NP).astype(np.float64)
    mid = (c64 - hi).astype(_BF16_NP).astype(np.float64)
    lo = (c64 - hi - mid).astype(_BF16_NP)
    return hi.astype(_BF16_NP), mid.astype(_BF16_NP), lo


def _corners(j0, i0, nj, ni):
    return np.array([[j0, i0, 1], [j0 + nj - 1, i0, 1],
                     [j0, i0 + ni - 1, 1], [j0 + nj - 1, i0 + ni - 1, 1]],
                    np.float64)


def _prepare(vertices, faces):
    planes, valid, xsmin, xsmax, ysmin, ysmax, zmin_tri = _planes64(vertices, faces)
    ntj = _W // _TJ
    ssj, ssi = _TJ // _SUBJ, _TI // _SUBI

    all_items = []
    for b in range(_B):
        P = planes[b]
        for t in range(_NTILE):
            tj, ti = t % ntj, t // ntj
            j0, i0 = tj * _TJ, ti * _TI
            xlo, xhi = 5 * j0 + 2.5, 5 * (j0 + _TJ - 1) + 2.5
            ylo, yhi = 5 * i0 + 2.5, 5 * (i0 + _TI - 1) + 2.5
            cand = np.where(valid[b] & (xsmax[b] >= xlo) & (xsmin[b] <= xhi)
                            & (ysmax[b] >= ylo) & (ysmin[b] <= yhi))[0]
            if len(cand) == 0:
                continue
            # exact subtile prune: keep a candidate iff it can win somewhere
            keep_any = np.zeros(len(cand), bool)
            Pk = [P[k][:, cand] for k in range(4)]
            zt = zmin_tri[b][cand]
            for sj in range(_SUBJ):
                for si in range(_SUBI):
                    C = _corners(j0 + sj * ssj, i0 + si * ssi, ssj, ssi)
                    alive = np.ones(len(cand), bool)
                    for k in range(3):
                        Pc = C @ Pk[k]
                        alive &= ~((Pc >= _OFF + _S * _REJ_MARGIN).all(axis=0))
                    idx = np.where(alive)[0]
                    if len(idx) == 0:
                        continue
                    Wc = C @ Pk[3][:, idx]
                    zlo = np.maximum(Wc.min(0), zt[idx])
                    covers = np.ones(len(idx), bool)
                    for k in range(3):
                        Pc = C @ Pk[k][:, idx]
                        covers &= (Pc <= _OFF - _S * _COVER_MARGIN).all(axis=0)
                    bound = (Wc.max(0)[covers].min() + _BOUND_MARGIN
                             ) if covers.any() else np.inf
                    keep_any[idx[zlo <= bound]] = True
            kept = cand[keep_any]
            if len(kept) == 0:
                continue
            Cf = _corners(j0, i0, _TJ, _TI)
            Wf = Cf @ P[3][:, kept]
            zlo_f = np.maximum(Wf.min(0), zmin_tri[b][kept])
            order = kept[np.argsort(zlo_f)]
            for c0 in range(0, len(order), _WMAX):
                all_items.append((b, t, order[c0 : c0 + _WMAX]))

    if not all_items:
        return None, None, None
    all_items.sort(key=lambda it: -len(it[2]))
    core_items = [[] for _ in range(8)]
    for r, it in enumerate(all_items):
        core_items[r % 8].append(it)

    nslot = max(len(ci) for ci in core_items)
    rawcaps = []
    for s in range(nslot):
        m = max((len(ci[s][2]) if s < len(ci) else 0) for ci in core_items)
        rawcaps.append(max(_GRAN, ((m + _GRAN - 1) // _GRAN) * _GRAN))

    if nslot % 2:
        rawcaps.append(_GRAN)
        nslot += 1
    groups = []
    s = 0
    while s < nslot:
        w = rawcaps[s]
        k = min(max(1, 512 // (2 * w)), nslot - s)
        if k > 1 and k % 2:
            k -= 1  # even k: 2x perf mode needs even dst elem count
        groups.append((w, k, s))
        s += k
    # processing order: two smallest groups first (the first group's
    # matmul+copy chain gates the first DVE op; the second's chain must
    # complete within the first group's DVE time or the DVE stalls), then
    # descending size.
    order = sorted(groups, key=lambda g: g[0] * g[1])
    sched = tuple(order[:2] + sorted(order[2:], key=lambda g: -g[0] * g[1]))
    xt, xe, colbase, pixe, pixo, cbB = _region_layout(sched)

    in_maps = []
    jl = (np.arange(128) % _TJ).astype(np.float32)
    il = (np.arange(128) // _TJ).astype(np.float32)
    pix1 = np.stack([jl, il, np.ones(128, np.float32)])
    pix9 = np.vstack([pix1, pix1, pix1]).astype(_BF16_NP)
    for c in range(8):
        items = core_items[c]
        stageA = np.zeros((3, xt), np.float64)
        stageB = np.zeros((3, xt), np.float64)
        stageA[2, :] = _BIGC
        stageB[2, :] = _BIGC
        for gi, (w, k, s0) in enumerate(sched):
            og = colbase[gi]
            for q in range(k):
                s = s0 + q
                if s < len(items):
                    b, t, idx = items[s]
                    n = len(idx)
                    tj, ti = t % ntj, t // ntj
                    j0, i0 = tj * _TJ, ti * _TI
                    Pl = np.empty((4, 3, n))
                    for kk in range(4):
                        a = planes[b, kk, 0, idx]
                        b_ = planes[b, kk, 1, idx]
                        cc = planes[b, kk, 2, idx] + a * j0 + b_ * i0
                        Pl[kk] = np.stack([a, b_, cc])
                    cA = og + 2 * w * q
                    stageA[:, cA : cA + n] = Pl[0]          # P0
                    stageA[:, cA + w : cA + w + n] = Pl[3]  # W
                    sB, cB = (stageA, cbB[gi] + 2 * w * q) if gi in cbB else (stageB, cA)
                    sB[:, cB : cB + n] = Pl[1]          # P1
                    sB[:, cB + w : cB + w + n] = Pl[2]  # P2
        data = np.zeros((18, xt), _BF16_NP)
        for rbase, stage in ((0, stageA), (9, stageB)):
            hi, mid, lo = _split3(stage)
            data[rbase : rbase + 9] = np.concatenate([hi, mid, lo], axis=0)
        for px in {pixe, pixo}:
            data[0:9, px : px + 128] = pix9
            data[9:18, px : px + 128] = pix9
        in_maps.append({"data": data})
    return sched, in_maps, core_items


def kernel(vertices, faces):
    vertices = np.asarray(vertices)
    faces = np.asarray(faces)
    sched, in_maps, core_items = _prepare(vertices, faces)
    if sched is None:
        return np.full((_B, _H, _W), _CLAMP, np.float32)

    nc = _get_nc(sched)
    kw = dict(PROFILE.get("run_kwargs", {}))
    res = run_bass_kernel_spmd(nc, in_maps, list(range(8)), **kw)
    PROFILE["last_result"] = res

    # device zmin columns are in processing order; map rank -> column
    rank2col = {}
    z0 = 0
    for w, k, s0 in sched:
        for q in range(k):
            rank2col[s0 + q] = z0 + q
        z0 += k

    ntj = _W // _TJ
    out = np.full((_B, _H, _W), _CLAMP, np.float32)
    for c in range(8):
        zraw = np.asarray(res.results[c]["out"]).astype(np.float32)  # [128, nslot]
        z = np.empty_like(zraw)
        for s, col in rank2col.items():
            z[:, s] = zraw[:, col]
        for s, (b, t, idx) in enumerate(core_items[c]):
            tj, ti = t % ntj, t // ntj
            j0, i0 = tj * _TJ, ti * _TI
            blk = z[:, s].reshape(_TI, _TJ)
            out[b, i0 : i0 + _TI, j0 : j0 + _TJ] = np.minimum(
                out[b, i0 : i0 + _TI, j0 : j0 + _TJ], blk)
    return out
